# revision 52
# baseline (speedup 1.0000x reference)
"""Trainium2 Bass kernel for nn_CA_2568390443063.

PoolBlock (2x depthwise-conv3x3-s2 + BN + PReLU) -> channel-similarity
softmax -> out = sim^T @ x, data-parallel over batch (1 sample / core,
8 NeuronCores).

Baseline-derived (77.4us) with scheduler-safe deltas:
  - conv2's bias matmul is gone: the bias rides the ACT stage pass
    (exact fp32), dropping 64 PE DoubleRows plus the dual-fp8 bias
    slots and the `ones` rhs tile.
  - conv1's 5th DR pair is (t8_hi, t8_lo) instead of (t8, zero-pad):
    same cost, one extra correction term.
  - the Exp activation table is pre-loaded via a dummy op right after
    the const DMAs, off the softmax critical path; the softmax -max
    scaling moved from ACT to DVE (same queue as the reduce, one
    cross-engine hop less).
  - pf tiles stay fp16 end-to-end: the PSUM->SBUF copies run in DVE's
    2x packed mode (192ns vs 258ns for the old fp8-pair copies, split
    12 ACT / 20 DVE) and S accumulates from fp16 tiles (32 matmuls at
    1 cyc/col) - better S precision and ~2us less ACT/DVE load in
    phase 1, which is three-way balanced (ACT 87% / PE 86% / DVE 85%).
  - phase-3 drains split ACT 35 / DVE 29 (ACT is ~15% faster/elem).

Kept from the baseline (scheduler constraints, learned the hard way):
pf transposes stay ON PE + ACT/DVE PSUM copies - the Tile wait
assigner snapshots cross-engine waits at scheduled-time queue
positions and HWDGE DMAs share 8 rings with the SP x-chunk stream, so
dma-transposes anywhere during the x-load inherit multi-us false
waits.

  - phase-boundary latency fixes (found by op-level trace reading):
    the T1 tail's last pt copies ride DVE (192ns vs 292 on ACT, which
    was serializing the final S chain), and the phase-3 flush drains
    unevenly (ACT 576 / DVE 448) with the last DMA on SP - the kernel
    exit waits the final DMA completion and Pool's SWDGE init is
    ~170ns slower than SP's HWDGE.

Measured (CoreSim cost model + 8-core run): 76587 ns, rel err
~3.9e-3 (tolerance 2e-2); baseline was 77385 ns at 6.2e-3.
"""
import sys
import numpy as np

sys.path.insert(0, "/opt/trn_rl_repo")

import ml_dtypes  # noqa: E402
import concourse.tile as tile  # noqa: E402
from concourse import bacc, mybir  # noqa: E402
from concourse.ap import AP  # noqa: E402

EPS = 1e-5
P = 128          # channels == SBUF partitions
H = W = 256
HP, WP = 258, 260            # padded x plane: x[r,c] at plane[r+1, c+2]
# hi/lo planes are interleaved ROW by ROW (hi row r, lo row r, hi r+1,
# ...) so the DoubleRow k-tile delta (260) fits the 16-bit ISA step
# field; row pitch is 2*WP
RPITCH = 2 * WP
PLANE = HP * WP
H1 = W1 = 128    # after conv1 (stride 2)
H2 = W2 = 64     # after conv2
N2 = H2 * W2     # 4096
NX = H * W       # 65536
HR = 132         # h1 window row stride; h1[r,c] at window col c+2
WROWS = 17       # window rows: h1 rows 16g-1 .. 16g+15
SIMSCALE = 128.0
F8 = ml_dtypes.float8_e4m3

f32 = mybir.dt.float32
f16 = mybir.dt.float16
f8 = mybir.dt.float8e4
AF = mybir.ActivationFunctionType
ALU = mybir.AluOpType
DRM = mybir.MatmulPerfMode.DoubleRow

TAPS = [(dy, dx) for dy in (-1, 0, 1) for dx in (-1, 0, 1)]
# conv1 slots [t0..t8, t8]; the 5th DR pair contracts (x_hi[t8], x_lo[t8])
# against (w8, w8) - the hi->lo k-tile delta within a row is +WP
PAIRS1 = [(0, 1), (2, 3), (4, 5), (6, 7), (8, 'lo8')]
# conv2 slots.  Bias is exact via the ACT stage pass (no bias matmul).
# W2_DUAL=False drops the lo weight plane: conv2 falls from 9 to 5
# DoubleRows per row, cutting ~107ns/group off the PE queue (the T1
# pace-setter).  rel err ~4e-3 -> ~1.2e-2 (tolerance 2e-2).
W2_DUAL = True
if W2_DUAL:
    PAIRS2 = [(('h', 0), ('h', 1)), (('h', 2), ('h', 3)),
              (('h', 4), ('h', 5)), (('h', 6), ('h', 7)),
              (('l', 0), ('l', 1)), (('l', 2), ('l', 3)),
              (('l', 4), ('l', 5)), (('l', 6), ('l', 8)),
              (('l', 7), ('h', 8))]
else:
    PAIRS2 = [(('h', 0), ('h', 1)), (('h', 2), ('h', 3)),
              (('h', 4), ('h', 5)), (('h', 6), ('h', 7)),
              (('h', 8), None)]
NSLOT1 = 10
NSLOT2 = 2 * len(PAIRS2)

# pt-copy index -> engine: 12 of 32 on ACT, rest on DVE (phase-1 balance)
PT_ACT = {t for t in range(32) if (t * 12) // 32 != ((t + 1) * 12) // 32}

# phase-3 drain chunks handled by ACT (the rest go to DVE); ACT's
# PSUM-source copies are ~15% cheaper so it takes 34 of 64.  (ACT and
# DVE are the ONLY options: the hw verifier rejects GpSimd/Pool PSUM
# reads, and SP has no compute.)
ACT_DRAIN = {k for k in range(64) if (k * 35) // 64 != ((k + 1) * 35) // 64}

# input DMA chunk row ranges [start, end) of each padded plane
CHUNKS = [(0, 9)] + [(8 * m + 1, 8 * m + 9) for m in range(1, 31)] + [(249, 258)]


def build_nc():
    nc = bacc.Bacc("TRN2", target_bir_lowering=False, debug=False,
                   enable_asserts=True, num_devices=8)

    x_d = nc.dram_tensor("xin", [P, 2 * PLANE], f8, kind="ExternalInput")
    w1_d = nc.dram_tensor("w1d", [P, NSLOT1 * P], f8, kind="ExternalInput")
    w2_d = nc.dram_tensor("w2d", [P, NSLOT2 * P], f8, kind="ExternalInput")
    prm_d = nc.dram_tensor("prm", [P, 4], f32, kind="ExternalInput")
    idn_d = nc.dram_tensor("idn", [P, P], f16, kind="ExternalInput")
    out_d = nc.dram_tensor("out", [P, NX], f16, kind="ExternalOutput")

    with tile.TileContext(nc) as tc:
        _emit(nc, tc, x_d, w1_d, w2_d, prm_d, idn_d, out_d)
    nc.compile()
    return nc


def _emit(nc, tc, x_d, w1_d, w2_d, prm_d, idn_d, out_d):
    from contextlib import ExitStack

    with ExitStack() as ctx:
        ep = ctx.enter_context
        consts = ep(tc.tile_pool(name="consts", bufs=1))
        cachep = ep(tc.tile_pool(name="cache", bufs=1))
        simp = ep(tc.tile_pool(name="simp", bufs=1))
        ptp = ep(tc.tile_pool(name="pt", bufs=10))
        ostp = ep(tc.tile_pool(name="ost", bufs=8))

        # ---- constants (ACT queue; SP/Pool are saturated with x chunks)
        w1sb = consts.tile([P, NSLOT1, P], f8)
        prm = consts.tile([P, 4], f32)
        scratch = consts.tile([P, 1], f32)
        w2sb = consts.tile([P, NSLOT2, P], f8)
        idn = consts.tile([P, P], f16)
        # scratch is memset (DVE) so the Exp table preload below need
        # not wait for any DMA-completion semaphore
        nc.vector.memset(scratch[:, :], 0.0)
        nc.scalar.dma_start(out=w1sb[:, :, :], in_=w1_d[:, :].rearrange(
            "p (t c) -> p t c", c=P))
        nc.scalar.dma_start(out=prm[:, :], in_=prm_d[:, :])
        # pre-load the Exp act table off both the softmax critical path
        # AND the first-y1 queue position (it reads scratch, not prm)
        nc.scalar.activation(scratch[:, :], scratch[:, :], AF.Exp,
                             bias=0.0, scale=1.0)

        def late_consts():
            # w2/idn aren't needed until the first conv2 block (~6us); slot
            # them on SP behind the first few x chunks
            nc.sync.dma_start(out=w2sb[:, :, :], in_=w2_d[:, :].rearrange(
                "p (t c) -> p t c", c=P))
            nc.sync.dma_start(out=idn[:, :], in_=idn_d[:, :])
        a1 = prm[:, 0:1]
        a2 = prm[:, 1:2]
        b1 = prm[:, 2:3]
        b2 = prm[:, 3:4]

        # ---- x planes, row-interleaved hi/lo
        xch = cachep.tile([P, 2 * PLANE], f8)
        for m, (ra, rb) in enumerate(CHUNKS):
            lo_el, hi_el = ra * RPITCH, rb * RPITCH
            if m == 0:
                # split the first chunk finer so conv1 group 0's first
                # rows start as soon as plane rows 0-2 land (the 900ns
                # DMA-completion semaphore dominates small transfers;
                # row-level Tile range tracking gates each output row)
                c1_, c2_ = 3 * RPITCH, 7 * RPITCH
                nc.sync.dma_start(out=xch[:, lo_el:c1_],
                                  in_=x_d[:, lo_el:c1_])
                nc.gpsimd.dma_start(out=xch[:, c1_:c2_],
                                    in_=x_d[:, c1_:c2_])
                nc.sync.dma_start(out=xch[:, c2_:hi_el],
                                  in_=x_d[:, c2_:hi_el])
                continue
            eng = (nc.sync, nc.gpsimd)[m % 2]
            eng.dma_start(out=xch[:, lo_el:hi_el], in_=x_d[:, lo_el:hi_el])
            if m == 3:
                late_consts()

        xap = xch[:, :]
        xt, xoff, xps = xap.tensor, xap.offset, xap.ap[0][0]

        def x_rhs(i, ta, tb):
            dya, dxa = TAPS[ta]
            base = (2 * i + dya + 1) * RPITCH + (dxa + 2)
            if tb == 'lo8':
                delta = WP  # same tap, lo plane (interleaved at +WP)
            else:
                delta = (TAPS[tb][0] - dya) * RPITCH + (TAPS[tb][1] - dxa)
            return AP(xt, xoff + base, [[xps, P], [delta, 2], [2, W1]])

        # ================= phase 1: convs + similarity accumulation
        pts = []
        with ExitStack() as ph1:
            h1wp = ph1.enter_context(tc.tile_pool(name="h1w", bufs=3))
            ybufp = ph1.enter_context(tc.tile_pool(name="ybuf", bufs=3))
            y2p = ph1.enter_context(tc.tile_pool(name="y2", bufs=3))
            h2p = ph1.enter_context(tc.tile_pool(name="h2b", bufs=2))
            cpsum = ph1.enter_context(
                tc.tile_pool(name="cpsum", bufs=4, space="PSUM"))
            tpsum = ph1.enter_context(
                tc.tile_pool(name="tpsum", bufs=2, space="PSUM"))
            spsum = ph1.enter_context(
                tc.tile_pool(name="spsum", bufs=1, space="PSUM"))

            def new_window(g):
                w_ = h1wp.tile([P, WROWS, HR], f8, tag="h1w")
                nc.vector.memset(w_[:, :, 0:2], 0.0)  # left halo (col 1)
                if not W2_DUAL:
                    # the conv2 pad k-tile (delta=1) reads through col 130
                    nc.vector.memset(w_[:, :, 130:132], 0.0)
                if g == 0:
                    nc.vector.memset(w_[:, 0, :], 0.0)  # h1 row -1
                return w_

            # PE p-state warm-up: the first x chunk takes ~3us to land; run
            # throwaway matmuls on a zeroed tile so the 3us ramp window is
            # spent before real conv work arrives
            warm = ybufp.tile([P, 4, W1], f16, tag="warm")
            nc.vector.memset(warm[:, 0, :], 0.0)
            wps = cpsum.tile([P, 4, W1], f32, tag="cps")
            for _ in range(20):
                nc.tensor.matmul(wps[:, 0, :], warm[:, 0, :], warm[:, 0, :],
                                 start=True, stop=True, skip_group_check=True)

            S = spsum.tile([P, P], f32)
            wins = {0: new_window(0)}
            # deferred PE stages: conv2 block g emits its transposes one
            # conv1 group later and its S matmuls two groups later, so the
            # in-order PE queue never waits on the DVE/ACT epilogue chain
            sched = {}

            def conv2_block(g2):
                c2 = cpsum.tile([P, 8, W2], f32, tag="cps")
                _conv2_rows(nc, c2, wins[g2], w2sb, range(8), True)
                wins.pop(g2)
                # stage pass with exact fp32 bias on ACT, then PReLU on
                # DVE (hw: stt may read only one PSUM operand)
                y2 = y2p.tile([P, 8, W2], f16, tag="y2")
                nc.scalar.activation(y2[:, :, :], c2[:, :, :], AF.Identity,
                                     bias=b2, scale=1.0)
                h2b = h2p.tile([P, 8 * W2], f16)
                nc.vector.scalar_tensor_tensor(
                    out=h2b[:, :].rearrange("p (a b) -> p a b", b=W2),
                    in0=y2[:, :, :], scalar=a2, in1=y2[:, :, :],
                    op0=ALU.mult, op1=ALU.max)

                def stage_tr():
                    # pt tiles stay fp16: the PSUM->SBUF copy then runs
                    # in DVE's 2x packed mode (192ns vs 258 for fp8
                    # out), and fp16 S is also more accurate
                    tiles = []
                    for q in range(4):
                        t = 4 * g2 + q
                        tp = tpsum.tile([P, P], f16)
                        nc.tensor.transpose(
                            tp[:, :], h2b[:, q * P:(q + 1) * P], idn[:, :])
                        pt = ptp.tile([P, P], f16, tag="ptp")
                        ceng = (nc.vector.tensor_copy,
                                nc.scalar.copy)[t in PT_ACT]
                        ceng(pt[:, :], tp[:, :])
                        tiles.append(pt)
                    pts.append(tiles)

                def stage_s():
                    for q, pt in enumerate(pts.pop(0)):
                        t = 4 * g2 + q
                        nc.tensor.matmul(S[:, :], pt[:, :], pt[:, :],
                                         start=(t == 0), stop=(t == 31),
                                         skip_group_check=True)
                return stage_tr, stage_s

            for m in range(32):
                # conv1 group m: output rows 4m..4m+3, one row per DR set
                c1 = cpsum.tile([P, 4, W1], f32, tag="cps")
                for r in range(4):
                    i = 4 * m + r
                    for d, (ta, tb) in enumerate(PAIRS1):
                        nc.tensor.matmul(
                            c1[:, r, :], w1sb[:, 2 * d:2 * d + 2, :],
                            x_rhs(i, ta, tb), start=(r == 0 and d == 0),
                            stop=(r == 3 and d == 4), perf_mode=DRM,
                            skip_group_check=True)
                for th in sched.pop(m, ()):
                    th()
                # epilogue: exact bias on ACT, then all-SBUF PReLU on DVE
                y1 = ybufp.tile([P, 4, W1], f16, tag="y1")
                nc.scalar.activation(y1[:, :, :], c1[:, :, :], AF.Identity,
                                     bias=b1, scale=1.0)
                loc = 4 * (m % 4) + 1
                nc.vector.scalar_tensor_tensor(
                    out=wins[m // 4][:, loc:loc + 4, 2:2 + W1],
                    in0=y1[:, :, :], scalar=a1, in1=y1[:, :, :],
                    op0=ALU.mult, op1=ALU.max)
                if m % 4 == 3 and m < 31:
                    # last h1 row of this window is also row -1 of the next
                    wins[m // 4 + 1] = new_window(m // 4 + 1)
                    nc.vector.scalar_tensor_tensor(
                        out=wins[m // 4 + 1][:, 0:1, 2:2 + W1],
                        in0=y1[:, 3:4, :], scalar=a1, in1=y1[:, 3:4, :],
                        op0=ALU.mult, op1=ALU.max)
                if m >= 4 and m % 4 == 0:
                    tr, s = conv2_block((m - 4) // 4)
                    sched.setdefault(m + 1, []).append(tr)
                    sched.setdefault(m + 2, []).append(s)
                if m == 30:
                    # final conv2 block: rows 0..5 only need h1 rows <= 125,
                    # available after group 30's epilogue - emit them at 31
                    # so only rows 6..7 trail the last conv1 group
                    def final_rows_early():
                        c2 = cpsum.tile([P, 8, W2], f32, tag="cps")
                        _conv2_rows(nc, c2, wins[7], w2sb, range(0, 6), True)
                        c2_final[0] = c2
                    c2_final = [None]
                    sched.setdefault(31, []).append(final_rows_early)
            # final block tail: epilogue rows 0-5 and their transposes /
            # S matmuls run as soon as the early rows' PSUM is ready; only
            # rows 6-7 wait on group 31's epilogue chain
            c2f = c2_final[0]
            win7 = wins.pop(7)
            y2f = y2p.tile([P, 8, W2], f16, tag="y2")
            h2b = h2p.tile([P, 8 * W2], f16)
            h2v = h2b[:, :].rearrange("p (a b) -> p a b", b=W2)
            nc.scalar.activation(y2f[:, 0:6, :], c2f[:, 0:6, :], AF.Identity,
                                 bias=b2, scale=1.0)
            nc.vector.scalar_tensor_tensor(
                out=h2v[:, 0:6, :], in0=y2f[:, 0:6, :], scalar=a2,
                in1=y2f[:, 0:6, :], op0=ALU.mult, op1=ALU.max)
            pts7 = []
            for q in range(3):
                tp = tpsum.tile([P, P], f16)
                nc.tensor.transpose(tp[:, :], h2b[:, q * P:(q + 1) * P],
                                    idn[:, :])
                pt = ptp.tile([P, P], f16, tag="ptp")
                # only the first copy on ACT: the tail ends with copies
                # back-to-back and DVE's (192ns, 2x packed) are faster
                (nc.scalar.copy, nc.vector.tensor_copy)[min(q, 1)](pt[:, :],
                                                                   tp[:, :])
                pts7.append(pt)
            _conv2_rows(nc, c2f, win7, w2sb, range(6, 8), False)
            nc.scalar.activation(y2f[:, 6:8, :], c2f[:, 6:8, :], AF.Identity,
                                 bias=b2, scale=1.0)
            nc.vector.scalar_tensor_tensor(
                out=h2v[:, 6:8, :], in0=y2f[:, 6:8, :], scalar=a2,
                in1=y2f[:, 6:8, :], op0=ALU.mult, op1=ALU.max)
            # transpose 3 and its copy are emitted BEFORE the S matmuls
            # for tiles 0-2, so PE's in-order queue reaches it at its
            # data dependency instead of behind three copy waits
            tp = tpsum.tile([P, P], f16)
            nc.tensor.transpose(tp[:, :], h2b[:, 3 * P:4 * P], idn[:, :])
            pt3 = ptp.tile([P, P], f16, tag="ptp")
            nc.vector.tensor_copy(pt3[:, :], tp[:, :])
            for q in range(3):
                nc.tensor.matmul(S[:, :], pts7[q][:, :], pts7[q][:, :],
                                 start=False, stop=False,
                                 skip_group_check=True)
            nc.tensor.matmul(S[:, :], pt3[:, :], pt3[:, :],
                             start=False, stop=True, skip_group_check=True)

            # ============= phase 2: softmax, sim -> scaled dual-fp8
            scale = float(N2) ** -0.5
            smallp = ph1.enter_context(tc.tile_pool(name="small", bufs=1))
            mx = smallp.tile([P, 1], f32)
            mb = smallp.tile([P, 1], f32)
            den = smallp.tile([P, 1], f32)
            rcp = smallp.tile([P, 1], f32)
            rs = smallp.tile([P, 1], f32)
            E = smallp.tile([P, P], f32)
            simhi2 = simp.tile([P, 2, P], f8)
            simlo2 = simp.tile([P, 2, P], f8)
            nc.vector.reduce_max(mx[:, :], S[:, :], axis=mybir.AxisListType.X)
            # -scale*max on DVE: same queue as the reduce, no extra hop
            nc.vector.tensor_scalar_mul(mb[:, :], mx[:, :], -scale)
            nc.scalar.activation(E[:, :], S[:, :], AF.Exp, bias=mb[:, :],
                                 scale=scale, accum_out=den[:, :])
            nc.vector.reciprocal(rcp[:, :], den[:, :])
            nc.vector.tensor_scalar_mul(rs[:, :], rcp[:, :], SIMSCALE)
            # write both DoubleRow k-tile copies at once via a stride-0
            # broadcast of E over the k-tile dim
            eap = E[:, :]
            ebc = AP(eap.tensor, eap.offset, [[eap.ap[0][0], P], [0, 2], [1, P]])
            nc.vector.tensor_scalar_mul(simhi2[:, :, :], ebc, rs[:, :])
            nc.vector.scalar_tensor_tensor(
                out=simlo2[:, :, :], in0=ebc, scalar=rs,
                in1=simhi2[:, :, :], op0=ALU.mult, op1=ALU.subtract)

        # ================= phase 3: out = sim^T @ (x_hi + x_lo)
        # 64 chunks of 1024 outputs through a 4-deep PSUM ring; drains
        # split ACT/DVE so both stream back-to-back
        inv = 1.0 / SIMSCALE
        with tc.tile_pool(name="opsum", bufs=4, space="PSUM") as opsum:
            for k in range(64):
                op = opsum.tile([P, 1024], f32)
                for rr in range(4):
                    row = 4 * k + rr
                    rhs = AP(xt, xoff + (row + 1) * RPITCH + 2,
                             [[xps, P], [WP, 2], [1, W]])
                    o = op[:, rr * W:(rr + 1) * W]
                    nc.tensor.matmul(o, simhi2[:, :, :], rhs,
                                     start=(rr % 2 == 0), stop=False,
                                     perf_mode=DRM, skip_group_check=True)
                    nc.tensor.matmul(o, simlo2[:, :, :], rhs,
                                     start=False, stop=(rr % 2 == 1),
                                     perf_mode=DRM, skip_group_check=True)
                ost = ostp.tile([P, 1024], f16)
                if k >= 62:
                    # tail: split the drain across both engines (uneven,
                    # 576/448, so both streams end together given their
                    # offset start times) and both DMA queues; the
                    # last-finishing drain's DMA rides SP (HWDGE init is
                    # ~170ns shorter than Pool's SWDGE, and the kernel
                    # exit waits on the final DMA completion)
                    nc.scalar.activation(ost[:, 0:576], op[:, 0:576],
                                         AF.Identity, bias=0.0, scale=inv)
                    nc.vector.tensor_scalar_mul(ost[:, 576:1024],
                                                op[:, 576:1024], inv)
                    nc.gpsimd.dma_start(
                        out=out_d[:, 1024 * k:1024 * k + 576],
                        in_=ost[:, 0:576])
                    nc.sync.dma_start(
                        out=out_d[:, 1024 * k + 576:1024 * (k + 1)],
                        in_=ost[:, 576:1024])
                    continue
                if k in ACT_DRAIN:
                    nc.scalar.activation(ost[:, :], op[:, :], AF.Identity,
                                         bias=0.0, scale=inv)
                else:
                    nc.vector.tensor_scalar_mul(ost[:, :], op[:, :], inv)
                oeng = (nc.sync, nc.gpsimd)[k % 2]
                oeng.dma_start(out=out_d[:, 1024 * k:1024 * (k + 1)],
                               in_=ost[:, :])


def _conv2_rows(nc, c2, win, w2sb, rows, start_first):
    wap = win[:, :, :]
    wt, woff, wps = wap.tensor, wap.offset, wap.ap[0][0]
    first = rows[0] if start_first else None
    for r in rows:
        for d, (ka, kb) in enumerate(PAIRS2):
            ta = ka[1]
            dya, dxa = TAPS[ta]
            base = (2 * r + dya + 1) * HR + (dxa + 2)
            if kb is None:
                delta = 1  # zero-weight pad slot; reads in-bounds garbage
            else:
                delta = (TAPS[kb[1]][0] - dya) * HR + (TAPS[kb[1]][1] - dxa)
            rhs = AP(wt, woff + base, [[wps, P], [delta, 2], [2, W2]])
            nc.tensor.matmul(c2[:, r, :], w2sb[:, 2 * d:2 * d + 2, :], rhs,
                             start=(r == first and d == 0),
                             stop=(r == 7 and d == len(PAIRS2) - 1),
                             perf_mode=DRM, skip_group_check=True)


def _q8(a):
    return np.asarray(a, np.float32).astype(F8)


def _diag_slots(cols):
    """cols: (P, nslot) fp32 -> (P, nslot, P) fp8 diagonal slot matrix."""
    nslot = cols.shape[1]
    d = np.zeros((P, nslot, P), np.float32)
    d[np.arange(P), :, np.arange(P)] = cols
    return d.astype(F8)


def _prep_params(inputs):
    """Host-side: fold BN into conv weights, build fp8 slot matrices."""
    def fold(w, gamma, beta, mean, var):
        inv = (np.asarray(gamma, np.float32)
               / np.sqrt(np.asarray(var, np.float32) + EPS))
        wf = (np.asarray(w, np.float32)[:, 0] * inv[:, None, None])
        b = (np.asarray(beta, np.float32)
             - np.asarray(mean, np.float32) * inv).astype(np.float32)
        return wf.reshape(P, 9), b

    w1f, b1 = fold(inputs["conv1_w"], inputs["bn1_gamma"], inputs["bn1_beta"],
                   inputs["bn1_mean"], inputs["bn1_var"])
    w2f, b2 = fold(inputs["conv2_w"], inputs["bn2_gamma"], inputs["bn2_beta"],
                   inputs["bn2_mean"], inputs["bn2_var"])

    def dual(v):
        hi = _q8(v)
        lo = _q8(v - hi.astype(np.float32))
        return hi.astype(np.float32), lo.astype(np.float32)

    # conv1 slots: [t0..t8 (hi), t8 again (pairs with the x_lo plane)]
    s1 = np.zeros((P, NSLOT1), np.float32)
    s1[:, 0:9] = _q8(w1f).astype(np.float32)
    s1[:, 9] = s1[:, 8]

    # conv2 slots: pairs from PAIRS2 (no bias slots; bias is on ACT)
    w2h, w2l = dual(w2f)
    s2 = np.zeros((P, NSLOT2), np.float32)
    for d, (ka, kb) in enumerate(PAIRS2):
        for j, k in enumerate((ka, kb)):
            if k is not None:
                s2[:, 2 * d + j] = (w2h if k[0] == 'h' else w2l)[:, k[1]]

    prm = np.stack([np.asarray(inputs["prelu1_a"], np.float32),
                    np.asarray(inputs["prelu2_a"], np.float32),
                    b1, b2], axis=1).astype(np.float32)
    return {
        "w1d": _diag_slots(s1).reshape(P, NSLOT1 * P),
        "w2d": _diag_slots(s2).reshape(P, NSLOT2 * P),
        "prm": prm,
        "idn": np.eye(P, dtype=np.float16),
    }


def _prep_x(xall):
    """(B, P, H, W) fp32 -> (B, P, 2*PLANE) fp8 row-interleaved planes."""
    B = xall.shape[0]
    planes = np.zeros((B, P, HP, 2, WP), F8)
    hi = xall.astype(F8)
    planes[:, :, 1:257, 0, 2:258] = hi
    planes[:, :, 1:257, 1, 2:258] = (xall - hi.astype(np.float32)).astype(F8)
    return planes.reshape(B, P, 2 * PLANE)


def sim_feed(inputs, shared=None):
    """Feed dict for CoreSim runs of core 0 (used by test.py)."""
    if shared is None:
        shared = _prep_params(inputs)
    x0 = np.asarray(inputs["x"][0:1], np.float32)
    return dict(shared, xin=_prep_x(x0)[0])


_nc_cache = {}


def get_nc():
    if "nc" not in _nc_cache:
        _nc_cache["nc"] = build_nc()
    return _nc_cache["nc"]


def kernel(**inputs) -> np.ndarray:
    from concourse.bass_utils import run_bass_kernel_spmd

    x = np.asarray(inputs["x"], np.float32)
    B = x.shape[0]
    shared = _prep_params(inputs)
    planes = _prep_x(x)
    in_maps = [dict(shared, xin=np.ascontiguousarray(planes[b]))
               for b in range(B)]
    nc = get_nc()
    res = run_bass_kernel_spmd(nc, in_maps, list(range(B)))
    out = np.stack([np.asarray(res.results[b]["out"], np.float16)
                    .astype(np.float32).reshape(P, H, W) for b in range(B)])
    return out
